# revision 2
# baseline (speedup 1.0000x reference)
"""GeneSAGE v2 on 8 trn2 cores — overlap-first design, f32 numerics.

Structure vs baseline:
- 1/cnt folded into the one-hot build (tensor_scalar is_equal*mult with two
  per-partition AP scalars) -> no count matmuls, no reciprocal chain.
- segment-sum matmul emits the aggregate transposed (gathered rows as
  lhsT, one-hot as rhs) so it lands feature-major (= mean^T with the fold)
  and feeds mean @ W1l directly; x^T+ones row precomputed host-side ->
  no dense-phase transposes of x/mean.
- chunks ordered by window within each stream; the two streams' gather
  batches alternate on Q7 so window w's chunks (both streams) arrive
  early and the dense phase interleaves per window under the gather prep.
- all f32: the harness's rel-err floor (1e-3 denom) demands ~1e-5 abs.
"""

import numpy as np

import concourse.mybir as mybir
from concourse import bacc, bass, tile
from concourse.bass_utils import run_bass_kernel_spmd

F32 = mybir.dt.float32
I16 = mybir.dt.int16

N_CORES = 8
D = 64
HID = 256
OUT = 2
LN_EPS = 1e-5
B_CH = 32
SINGLE_PACKET = False


def make_plan(edge_index: np.ndarray, n_nodes: int):
    cp = int(np.ceil(n_nodes / (N_CORES * 128))) * 128
    nw = cp // 128
    npad = N_CORES * cp
    half = npad // 2
    assert half <= 32768

    src = edge_index[0].astype(np.int64)
    dst = edge_index[1].astype(np.int64)
    E = src.shape[0]

    core = dst // cp
    stream = (src >= half).astype(np.int64)
    win = (dst % cp) // 128
    ngrp = 2 * nw
    key = (core * 2 + stream) * nw + win
    order = np.argsort(key, kind="stable")
    counts = np.bincount(key, minlength=N_CORES * ngrp).reshape(
        N_CORES, 2, nw)
    nchunks = -(-counts.max(axis=0) // 128)  # [2, nw]
    off = np.zeros((2, nw), np.int64)
    running = 0
    for s in range(2):
        for w in range(nw):
            off[s, w] = running
            running += nchunks[s, w]
    c_total = int(running)
    c_lo = int(nchunks[0].sum())
    e_slots = c_total * 128

    sk = key[order]
    grp_start = np.searchsorted(sk, np.arange(N_CORES * ngrp))
    rank = np.arange(E) - grp_start[sk]
    s_of = (sk // nw) % 2
    w_of = sk % nw
    c_of = sk // ngrp
    slot = off[s_of, w_of] * 128 + rank

    cnt = np.bincount(dst, minlength=npad).astype(np.float32)
    rcnt = 1.0 / np.maximum(cnt, 1.0)

    gidx = np.zeros((N_CORES, e_slots), np.int16)
    dstf = np.full((N_CORES, e_slots), -1.0, np.float32)
    rcf = np.ones((N_CORES, e_slots), np.float32)
    srcslot = np.zeros((N_CORES, e_slots), np.int32)
    gidx[c_of, slot] = (src[order] - s_of * half).astype(np.int16)
    dstf[c_of, slot] = (dst[order] % 128).astype(np.float32)
    rcf[c_of, slot] = rcnt[dst[order]]
    srcslot[c_of, slot] = src[order].astype(np.int32)

    a = gidx.reshape(N_CORES, e_slots // 16, 16).transpose(0, 2, 1)
    gidx_tile = np.tile(a, (1, 8, 1)).copy()  # [c, 128, J]
    dstf_tile = np.ascontiguousarray(
        dstf.reshape(N_CORES, c_total, 128).transpose(0, 2, 1))
    rcf_tile = np.ascontiguousarray(
        rcf.reshape(N_CORES, c_total, 128).transpose(0, 2, 1))

    # per window: ordered chunk list [(chunk_id, stream), ...]
    wchunks = []
    for w in range(nw):
        rows = []
        for s in range(2):
            f = int(off[s, w])
            for g in range(f, f + int(nchunks[s, w])):
                rows.append((g, s))
        wchunks.append(rows)

    return dict(
        cp=cp, nw=nw, npad=npad, half=half, c_total=c_total, c_lo=c_lo,
        wchunks=wchunks, gidx_tile=gidx_tile, dstf_tile=dstf_tile,
        rcf_tile=rcf_tile, srcslot=srcslot, e_slots=e_slots,
    )


def build_program(plan):
    cp, nw, half = plan["cp"], plan["nw"], plan["half"]
    c_total, c_lo = plan["c_total"], plan["c_lo"]
    wchunks = plan["wchunks"]
    J = c_total * 8

    nc = bacc.Bacc("TRN2", target_bir_lowering=False, debug=False,
                   num_devices=N_CORES)

    def inp(name, shape, dt=F32):
        return nc.dram_tensor(name, shape, dt, kind="ExternalInput").ap()

    msg1_d = inp("msg1", [c_total * 128, D])
    gidx_d = inp("gidx", [128, J], I16)
    dstf_d = inp("dstf", [128, c_total])
    rcf_d = inp("rcf", [128, c_total])
    iota_d = inp("iota", [128, 128])
    ident_d = inp("ident", [128, 128])
    xt_d = inp("xt", [D + 1, cp])
    wcb_d = inp("wcb", [D + 1, HID])
    w1l_d = inp("w1l", [D, HID])
    w2lr_d = inp("w2lr", [128, 2 * 2 * OUT])
    gamma_d = inp("gamma_bc", [128, HID])
    beta_d = inp("beta_bc", [128, HID])
    b2_d = inp("b2_bc", [128, OUT])
    i2_d = inp("i2", [2, D])
    out_d = nc.dram_tensor("out", [cp, OUT], F32, kind="ExternalOutput").ap()

    ranges = [(0, c_lo), (c_lo, c_total)]

    with tile.TileContext(nc) as tc:
        with (
            tc.tile_pool(name="res", bufs=1) as res,
            tc.tile_pool(name="dram", bufs=1, space="DRAM") as dram,
        ):
            def load(name, ap, shape, dt=F32):
                t = res.tile(shape, dt, tag=name, name=name)
                nc.sync.dma_start(out=t[:], in_=ap)
                return t

            gidx_sb = load("gidx", gidx_d, [128, J], I16)
            dstf_sb = load("dstf", dstf_d, [128, c_total])
            rcf_sb = load("rcf", rcf_d, [128, c_total])
            iota_sb = load("iota", iota_d, [128, 128])
            ident_sb = load("ident", ident_d, [128, 128])
            xt_sb = load("xt", xt_d, [D + 1, cp])
            wcb_sb = load("wcb", wcb_d, [D + 1, HID])
            w1l_sb = load("w1l", w1l_d, [D, HID])
            w2lr_sb = load("w2lr", w2lr_d, [128, 2 * 2 * OUT])
            gamma_sb = load("gamma", gamma_d, [128, HID])
            beta_sb = load("beta", beta_d, [128, HID])
            b2_sb = load("b2", b2_d, [128, OUT])
            i2_sb = load("i2", i2_d, [2, D])

            pr_sb = res.tile([128, nw, 2 * OUT], F32, tag="prsb", name="prsb")
            out_sb = res.tile([128, nw, OUT], F32, tag="outsb", name="outsb")

            pb2_mine = dram.tile([cp, D], F32)
            pb2_full = dram.tile([N_CORES * cp, D], F32)

            def onehot(opool, g):
                o = opool.tile([128, 128], F32, tag="O")
                nc.vector.tensor_scalar(
                    out=o[:], in0=iota_sb[:],
                    scalar1=dstf_sb[:, g : g + 1],
                    scalar2=rcf_sb[:, g : g + 1],
                    op0=mybir.AluOpType.is_equal,
                    op1=mybir.AluOpType.mult,
                )
                return o

            def batch_ranges():
                per_stream = []
                for s in range(2):
                    lo, hi = ranges[s]
                    per_stream.append(
                        [(b0, min(b0 + B_CH, hi))
                         for b0 in range(lo, hi, B_CH)])
                out = []
                nb = max(len(per_stream[0]), len(per_stream[1]))
                for k in range(nb):
                    for s in range(2):
                        if k < len(per_stream[s]):
                            out.append((s, per_stream[s][k]))
                return out

            def stream_batches(pools):
                """Pass 1: host pre-permuted messages, plain strided DMA."""
                lookup = {}
                for s, (b0, b1) in batch_ranges():
                    g = pools[s].tile([128, B_CH, D], F32, tag="gbuf")
                    nc.sync.dma_start(
                        out=g[:, 0 : b1 - b0, :],
                        in_=msg1_d[b0 * 128 : b1 * 128, :].rearrange(
                            "(c p) d -> p c d", p=128))
                    for gg in range(b0, b1):
                        lookup[gg] = (g, gg - b0)
                return lookup

            def gather_batches(pools, tables):
                """Pass 2: Q7 dma_gather, alternating the two streams."""
                lookup = {}
                for s, (b0, b1) in batch_ranges():
                    g = pools[s].tile([128, B_CH, D], F32, tag="gbuf")
                    n_idx = (b1 - b0) * 128
                    nc.gpsimd.dma_gather(
                        out_ap=g[:, 0 : b1 - b0, :],
                        in_ap=tables[s],
                        idxs_ap=gidx_sb[:, b0 * 8 : b1 * 8],
                        num_idxs=n_idx,
                        num_idxs_reg=n_idx,
                        elem_size=D,
                        single_packet=SINGLE_PACKET,
                    )
                    for gg in range(b0, b1):
                        lookup[gg] = (g, gg - b0)
                return lookup

            # ============ pass 1: conv1 agg + dense, per window ============
            with (
                tc.tile_pool(name="gp0", bufs=4) as gp0,
                tc.tile_pool(name="gp1", bufs=4) as gp1,
                tc.tile_pool(name="opool", bufs=64) as opool,
                tc.tile_pool(name="pagg", bufs=2, space="PSUM") as pagg,
                tc.tile_pool(name="px1", bufs=2, space="PSUM") as px1,
                tc.tile_pool(name="ptp", bufs=1, space="PSUM") as ptp,
                tc.tile_pool(name="ppr", bufs=1, space="PSUM") as ppr,
                tc.tile_pool(name="dwork", bufs=3) as dwork,
                tc.tile_pool(name="dsmall", bufs=4) as dsmall,
                tc.tile_pool(name="ptpool", bufs=1) as ptpool,
                tc.tile_pool(name="bpp", bufs=1, space="PSUM") as bpp,
                tc.tile_pool(name="bstage", bufs=2) as bstage,
            ):
                lookup = stream_batches((gp0, gp1))
                pt_sb = ptpool.tile([2, cp], F32)
                STG = 7
                stage = None
                for w in range(nw):
                    rows = wchunks[w]
                    aggT = pagg.tile([D, 128], F32, tag="aggT")
                    for k, (g, s) in enumerate(rows):
                        gb, j = lookup[g]
                        o = onehot(opool, g)
                        nc.tensor.matmul(
                            aggT[:], gb[:, j, :], o[:],
                            start=(k == 0), stop=(k == len(rows) - 1),
                        )
                    meanT = dwork.tile([D, 128], F32, tag="meanT")
                    nc.scalar.activation(
                        meanT[:], aggT[:], mybir.ActivationFunctionType.Copy)

                    x1p = px1.tile([128, HID], F32, tag="x1")
                    nc.tensor.matmul(
                        x1p[:], xt_sb[:, 128 * w : 128 * (w + 1)], wcb_sb[:],
                        start=True, stop=False)
                    nc.tensor.matmul(
                        x1p[:], meanT[:], w1l_sb[:], start=False, stop=True)

                    mu = dsmall.tile([128, 1], F32, tag="mu")
                    nc.vector.reduce_sum(
                        out=mu[:], in_=x1p[:], axis=mybir.AxisListType.X)
                    nc.vector.tensor_scalar(
                        out=mu[:], in0=mu[:], scalar1=1.0 / HID,
                        scalar2=None, op0=mybir.AluOpType.mult)
                    xc = dwork.tile([128, HID], F32, tag="xc")
                    nc.vector.tensor_scalar(
                        out=xc[:], in0=x1p[:], scalar1=mu[:], scalar2=None,
                        op0=mybir.AluOpType.subtract)
                    sq = dwork.tile([128, HID], F32, tag="sq")
                    var = dsmall.tile([128, 1], F32, tag="var")
                    nc.scalar.activation(
                        sq[:], xc[:], mybir.ActivationFunctionType.Square,
                        accum_out=var[:])
                    rstd = dsmall.tile([128, 1], F32, tag="rstd")
                    nc.vector.tensor_scalar(
                        out=rstd[:], in0=var[:], scalar1=1.0 / HID,
                        scalar2=LN_EPS, op0=mybir.AluOpType.mult,
                        op1=mybir.AluOpType.add)
                    nc.scalar.activation(
                        rstd[:], rstd[:], mybir.ActivationFunctionType.Sqrt)
                    nc.vector.reciprocal(rstd[:], rstd[:])
                    y = dwork.tile([128, HID], F32, tag="y")
                    nc.vector.tensor_scalar(
                        out=y[:], in0=xc[:], scalar1=rstd[:], scalar2=None,
                        op0=mybir.AluOpType.mult)
                    nc.vector.tensor_tensor(
                        out=y[:], in0=y[:], in1=gamma_sb[:],
                        op=mybir.AluOpType.mult)
                    nc.vector.tensor_tensor(
                        out=y[:], in0=y[:], in1=beta_sb[:],
                        op=mybir.AluOpType.add)
                    m0 = dwork.tile([128, HID], F32, tag="m0")
                    nc.vector.tensor_scalar(
                        out=m0[:], in0=y[:], scalar1=0.0, scalar2=None,
                        op0=mybir.AluOpType.min)
                    ex = dwork.tile([128, HID], F32, tag="ex")
                    nc.scalar.activation(
                        ex[:], m0[:], mybir.ActivationFunctionType.Exp)
                    rm1 = dwork.tile([128, HID], F32, tag="rm1")
                    nc.vector.tensor_scalar(
                        out=rm1[:], in0=y[:], scalar1=0.0, scalar2=-1.0,
                        op0=mybir.AluOpType.max, op1=mybir.AluOpType.add)
                    h = dwork.tile([128, HID], F32, tag="h")
                    nc.vector.tensor_tensor(
                        out=h[:], in0=rm1[:], in1=ex[:],
                        op=mybir.AluOpType.add)

                    prp = ppr.tile([128, 2 * OUT], F32, tag="pr")
                    for hh in range(2):
                        tph = ptp.tile([128, 128], F32, tag="tph")
                        nc.tensor.transpose(
                            tph[:], h[:, 128 * hh : 128 * (hh + 1)],
                            ident_sb[:])
                        hts = dwork.tile([128, 128], F32, tag="hts")
                        nc.scalar.activation(
                            hts[:], tph[:], mybir.ActivationFunctionType.Copy)
                        nc.tensor.matmul(
                            prp[:], hts[:],
                            w2lr_sb[:, 4 * hh : 4 * (hh + 1)],
                            start=(hh == 0), stop=(hh == 1))
                    nc.scalar.activation(
                        pr_sb[:, w, :], prp[:],
                        mybir.ActivationFunctionType.Copy)
                    ptw = ptp.tile([OUT, 128], F32, tag="ptw")
                    nc.tensor.transpose(
                        ptw[:], pr_sb[:, w, 0:OUT], ident_sb[:])
                    nc.scalar.activation(
                        pt_sb[:, 128 * w : 128 * (w + 1)], ptw[:],
                        mybir.ActivationFunctionType.Copy)

                    # local replicated-p block for this window (hidden
                    # under pass-1 gathers); allgathered below
                    pp = bpp.tile([128, D], F32, tag="pb2p")
                    nc.tensor.matmul(
                        pp[:], pt_sb[:, 128 * w : 128 * (w + 1)],
                        i2_sb[:], start=True, stop=True)
                    if w % STG == 0:
                        stage = bstage.tile([128, STG, D], F32, tag="stage")
                    nc.scalar.activation(
                        stage[:, w % STG, :], pp[:],
                        mybir.ActivationFunctionType.Copy)
                    if w % STG == STG - 1:
                        w0 = w - STG + 1
                        nc.sync.dma_start(
                            out=pb2_mine[w0 * 128 : (w0 + STG) * 128, :]
                            .rearrange("(s p) d -> p s d", p=128),
                            in_=stage[:])

            # ============ replicated-p table all-gather ============
            nc.gpsimd.collective_compute(
                "AllGather",
                mybir.AluOpType.bypass,
                replica_groups=[list(range(N_CORES))],
                ins=[pb2_mine.opt()],
                outs=[pb2_full.opt()],
            )

            # ============ pass 2: conv2 agg + output, per window ============
            with (
                tc.tile_pool(name="g2p0", bufs=4) as g2p0,
                tc.tile_pool(name="g2p1", bufs=4) as g2p1,
                tc.tile_pool(name="opool2", bufs=64) as opool2,
                tc.tile_pool(name="pagg2", bufs=2, space="PSUM") as pagg2,
                tc.tile_pool(name="fsmall", bufs=4) as fsmall,
            ):
                lookup2 = gather_batches(
                    (g2p0, g2p1),
                    (pb2_full[0:half, :], pb2_full[half : 2 * half, :]))
                for w in range(nw):
                    rows = wchunks[w]
                    agg2 = pagg2.tile([128, OUT], F32, tag="agg2")
                    for k, (g, s) in enumerate(rows):
                        gb, j = lookup2[g]
                        o = onehot(opool2, g)
                        nc.tensor.matmul(
                            agg2[:], o[:], gb[:, j, 0:OUT],
                            start=(k == 0), stop=(k == len(rows) - 1),
                        )
                    t = fsmall.tile([128, OUT], F32, tag="fo")
                    nc.vector.tensor_tensor(
                        out=t[:], in0=agg2[:], in1=pr_sb[:, w, OUT : 2 * OUT],
                        op=mybir.AluOpType.add)
                    nc.vector.tensor_tensor(
                        out=out_sb[:, w, :], in0=t[:], in1=b2_sb[:],
                        op=mybir.AluOpType.add)
                    if w % 7 == 6:
                        w0 = w - 6
                        nc.sync.dma_start(
                            out=out_d.rearrange(
                                "(w p) c -> p w c", p=128)[:, w0 : w0 + 7, :],
                            in_=out_sb[:, w0 : w0 + 7, :])

    nc.compile()
    return nc


def make_inputs(plan, x, W1l, W1r, b1, Wskip, bskip, gamma, beta, W2l, W2r,
                b2, n_nodes):
    cp, half, npad = plan["cp"], plan["half"], plan["npad"]
    xp = np.zeros((npad, D), np.float32)
    xp[:n_nodes] = np.asarray(x, np.float32)
    wc = np.asarray(W1r, np.float32) + np.asarray(Wskip, np.float32)
    bc = np.asarray(b1, np.float32) + np.asarray(bskip, np.float32)
    wcb = np.concatenate([wc, bc[None, :]], axis=0)
    w2lr_full = np.concatenate(
        [np.asarray(W2l, np.float32), np.asarray(W2r, np.float32)], axis=1)
    w2lr = (w2lr_full.reshape(2, 128, 2 * OUT).transpose(1, 0, 2)
            .reshape(128, 2 * 2 * OUT).copy())
    iota = np.tile(np.arange(128, dtype=np.float32)[None, :], (128, 1))
    ident = np.eye(128, dtype=np.float32)
    i2 = np.zeros((2, D), np.float32)
    i2[0, 0::2] = 1.0
    i2[1, 1::2] = 1.0
    gamma_bc = np.tile(np.asarray(gamma, np.float32)[None, :], (128, 1))
    beta_bc = np.tile(np.asarray(beta, np.float32)[None, :], (128, 1))
    b2_bc = np.tile(np.asarray(b2, np.float32)[None, :], (128, 1))

    common = dict(
        iota=iota, ident=ident, wcb=wcb,
        w1l=np.asarray(W1l, np.float32), w2lr=w2lr,
        gamma_bc=gamma_bc, beta_bc=beta_bc, b2_bc=b2_bc, i2=i2,
    )
    in_maps = []
    for c in range(N_CORES):
        m = dict(common)
        xt = np.ones((D + 1, cp), np.float32)
        xt[:D] = xp[cp * c : cp * (c + 1)].T
        m["xt"] = xt
        m["msg1"] = xp[plan["srcslot"][c]]
        m["gidx"] = plan["gidx_tile"][c]
        m["dstf"] = plan["dstf_tile"][c]
        m["rcf"] = plan["rcf_tile"][c]
        in_maps.append(m)
    return in_maps


_CACHE = {}


def _get_compiled(edge_index, n_nodes):
    key = (edge_index.tobytes()[:512], edge_index.shape, n_nodes)
    if key not in _CACHE:
        plan = make_plan(edge_index, n_nodes)
        nc = build_program(plan)
        _CACHE[key] = (plan, nc)
    return _CACHE[key]


def run(inputs, trace=False):
    x = np.asarray(inputs["x"], np.float32)
    edge_index = np.asarray(inputs["edge_index"], np.int32)
    n_nodes = x.shape[0]
    plan, nc = _get_compiled(edge_index, n_nodes)
    in_maps = make_inputs(
        plan, x, inputs["W1l"], inputs["W1r"], inputs["b1"], inputs["Wskip"],
        inputs["bskip"], inputs["gamma"], inputs["beta"], inputs["W2l"],
        inputs["W2r"], inputs["b2"], n_nodes)
    res = run_bass_kernel_spmd(
        nc, in_maps, list(range(N_CORES)), trace=trace)
    cp = plan["cp"]
    out = np.empty((n_nodes, OUT), np.float32)
    for c in range(N_CORES):
        lo = cp * c
        hi = min(cp * (c + 1), n_nodes)
        out[lo:hi] = res.results[c]["out"][0 : hi - lo]
    return out, res


def kernel(**inputs) -> np.ndarray:
    out, _ = run(inputs)
    return out


# revision 3
# speedup vs baseline: 1.1614x; 1.1614x over previous
"""GeneSAGE v2 on 8 trn2 cores — overlap-first design, f32 numerics.

Structure vs baseline:
- 1/cnt folded into the one-hot build (tensor_scalar is_equal*mult with two
  per-partition AP scalars) -> no count matmuls, no reciprocal chain.
- segment-sum matmul emits the aggregate transposed (gathered rows as
  lhsT, one-hot as rhs) so it lands feature-major (= mean^T with the fold)
  and feeds mean @ W1l directly; x^T+ones row precomputed host-side ->
  no dense-phase transposes of x/mean.
- chunks ordered by window within each stream; the two streams' gather
  batches alternate on Q7 so window w's chunks (both streams) arrive
  early and the dense phase interleaves per window under the gather prep.
- all f32: the harness's rel-err floor (1e-3 denom) demands ~1e-5 abs.
"""

import numpy as np

import concourse.mybir as mybir
from concourse import bacc, bass, tile
from concourse.bass_utils import run_bass_kernel_spmd

F32 = mybir.dt.float32
I16 = mybir.dt.int16

N_CORES = 8
D = 64
HID = 256
OUT = 2
LN_EPS = 1e-5
B_CH = 16
SINGLE_PACKET = False


def make_plan(edge_index: np.ndarray, n_nodes: int):
    cp = int(np.ceil(n_nodes / (N_CORES * 128))) * 128
    nw = cp // 128
    npad = N_CORES * cp
    half = npad // 2
    assert half <= 32768

    src = edge_index[0].astype(np.int64)
    dst = edge_index[1].astype(np.int64)
    E = src.shape[0]

    core = dst // cp
    stream = (src >= half).astype(np.int64)
    win = (dst % cp) // 128
    ngrp = 2 * nw
    key = (core * 2 + stream) * nw + win
    order = np.argsort(key, kind="stable")
    counts = np.bincount(key, minlength=N_CORES * ngrp).reshape(
        N_CORES, 2, nw)
    nchunks = -(-counts.max(axis=0) // 128)  # [2, nw]
    off = np.zeros((2, nw), np.int64)
    running = 0
    for s in range(2):
        for w in range(nw):
            off[s, w] = running
            running += nchunks[s, w]
    c_total = int(running)
    c_lo = int(nchunks[0].sum())
    e_slots = c_total * 128

    sk = key[order]
    grp_start = np.searchsorted(sk, np.arange(N_CORES * ngrp))
    rank = np.arange(E) - grp_start[sk]
    s_of = (sk // nw) % 2
    w_of = sk % nw
    c_of = sk // ngrp
    slot = off[s_of, w_of] * 128 + rank

    cnt = np.bincount(dst, minlength=npad).astype(np.float32)
    rcnt = 1.0 / np.maximum(cnt, 1.0)

    gidx = np.zeros((N_CORES, e_slots), np.int16)
    dstf = np.full((N_CORES, e_slots), -1.0, np.float32)
    rcf = np.ones((N_CORES, e_slots), np.float32)
    srcslot = np.zeros((N_CORES, e_slots), np.int32)
    gidx[c_of, slot] = (src[order] - s_of * half).astype(np.int16)
    dstf[c_of, slot] = (dst[order] % 128).astype(np.float32)
    rcf[c_of, slot] = rcnt[dst[order]]
    srcslot[c_of, slot] = src[order].astype(np.int32)

    a = gidx.reshape(N_CORES, e_slots // 16, 16).transpose(0, 2, 1)
    gidx_tile = np.tile(a, (1, 8, 1)).copy()  # [c, 128, J]
    dstf_tile = np.ascontiguousarray(
        dstf.reshape(N_CORES, c_total, 128).transpose(0, 2, 1))
    rcf_tile = np.ascontiguousarray(
        rcf.reshape(N_CORES, c_total, 128).transpose(0, 2, 1))

    # per window: ordered chunk list [(chunk_id, stream), ...]
    wchunks = []
    for w in range(nw):
        rows = []
        for s in range(2):
            f = int(off[s, w])
            for g in range(f, f + int(nchunks[s, w])):
                rows.append((g, s))
        wchunks.append(rows)

    return dict(
        cp=cp, nw=nw, npad=npad, half=half, c_total=c_total, c_lo=c_lo,
        wchunks=wchunks, gidx_tile=gidx_tile, dstf_tile=dstf_tile,
        rcf_tile=rcf_tile, srcslot=srcslot, e_slots=e_slots,
    )


def build_program(plan):
    cp, nw, half = plan["cp"], plan["nw"], plan["half"]
    c_total, c_lo = plan["c_total"], plan["c_lo"]
    wchunks = plan["wchunks"]
    J = c_total * 8

    nc = bacc.Bacc("TRN2", target_bir_lowering=False, debug=False,
                   num_devices=N_CORES)

    def inp(name, shape, dt=F32):
        return nc.dram_tensor(name, shape, dt, kind="ExternalInput").ap()

    msg1_d = inp("msg1", [c_total * 128, D])
    gidx_d = inp("gidx", [128, J], I16)
    dstf_d = inp("dstf", [128, c_total])
    rcf_d = inp("rcf", [128, c_total])
    iota_d = inp("iota", [128, 128])
    ident_d = inp("ident", [128, 128])
    xt_d = inp("xt", [D + 1, cp])
    wcb_d = inp("wcb", [D + 1, HID])
    w1l_d = inp("w1l", [D, HID])
    w2lr_d = inp("w2lr", [128, 2 * 2 * OUT])
    gamma_d = inp("gamma_bc", [128, HID])
    beta_d = inp("beta_bc", [128, HID])
    b2_d = inp("b2_bc", [128, OUT])
    i2_d = inp("i2", [2, D])
    out_d = nc.dram_tensor("out", [cp, OUT], F32, kind="ExternalOutput").ap()

    ranges = [(0, c_lo), (c_lo, c_total)]

    with tile.TileContext(nc) as tc:
        with (
            tc.tile_pool(name="res", bufs=1) as res,
            tc.tile_pool(name="dram", bufs=1, space="DRAM") as dram,
        ):
            def load(name, ap, shape, dt=F32):
                t = res.tile(shape, dt, tag=name, name=name)
                nc.sync.dma_start(out=t[:], in_=ap)
                return t

            gidx_sb = load("gidx", gidx_d, [128, J], I16)
            dstf_sb = load("dstf", dstf_d, [128, c_total])
            rcf_sb = load("rcf", rcf_d, [128, c_total])
            iota_sb = load("iota", iota_d, [128, 128])
            ident_sb = load("ident", ident_d, [128, 128])
            xt_sb = load("xt", xt_d, [D + 1, cp])
            wcb_sb = load("wcb", wcb_d, [D + 1, HID])
            w1l_sb = load("w1l", w1l_d, [D, HID])
            w2lr_sb = load("w2lr", w2lr_d, [128, 2 * 2 * OUT])
            gamma_sb = load("gamma", gamma_d, [128, HID])
            beta_sb = load("beta", beta_d, [128, HID])
            b2_sb = load("b2", b2_d, [128, OUT])
            i2_sb = load("i2", i2_d, [2, D])

            pr_sb = res.tile([128, nw, 2 * OUT], F32, tag="prsb", name="prsb")
            out_sb = res.tile([128, nw, OUT], F32, tag="outsb", name="outsb")

            pb2_mine = dram.tile([cp, D], F32)
            pb2_full = dram.tile([N_CORES * cp, D], F32)

            def onehot(opool, g):
                o = opool.tile([128, 128], F32, tag="O")
                nc.vector.tensor_scalar(
                    out=o[:], in0=iota_sb[:],
                    scalar1=dstf_sb[:, g : g + 1],
                    scalar2=rcf_sb[:, g : g + 1],
                    op0=mybir.AluOpType.is_equal,
                    op1=mybir.AluOpType.mult,
                )
                return o

            def batch_ranges():
                per_stream = []
                for s in range(2):
                    lo, hi = ranges[s]
                    per_stream.append(
                        [(b0, min(b0 + B_CH, hi))
                         for b0 in range(lo, hi, B_CH)])
                out = []
                nb = max(len(per_stream[0]), len(per_stream[1]))
                for k in range(nb):
                    for s in range(2):
                        if k < len(per_stream[s]):
                            out.append((s, per_stream[s][k]))
                return out

            def stream_batches(pools):
                """Pass 1: host pre-permuted messages, plain strided DMA."""
                lookup = {}
                for s, (b0, b1) in batch_ranges():
                    g = pools[s].tile([128, B_CH, D], F32, tag="gbuf")
                    nc.sync.dma_start(
                        out=g[:, 0 : b1 - b0, :],
                        in_=msg1_d[b0 * 128 : b1 * 128, :].rearrange(
                            "(c p) d -> p c d", p=128))
                    for gg in range(b0, b1):
                        lookup[gg] = (g, gg - b0)
                return lookup

            def gather_batches(pools, tables):
                """Pass 2: Q7 dma_gather, alternating the two streams."""
                lookup = {}
                for s, (b0, b1) in batch_ranges():
                    g = pools[s].tile([128, B_CH, D], F32, tag="gbuf")
                    n_idx = (b1 - b0) * 128
                    nc.gpsimd.dma_gather(
                        out_ap=g[:, 0 : b1 - b0, :],
                        in_ap=tables[s],
                        idxs_ap=gidx_sb[:, b0 * 8 : b1 * 8],
                        num_idxs=n_idx,
                        num_idxs_reg=n_idx,
                        elem_size=D,
                        single_packet=SINGLE_PACKET,
                    )
                    for gg in range(b0, b1):
                        lookup[gg] = (g, gg - b0)
                return lookup

            # ============ pass 1: conv1 agg + dense, per window ============
            with (
                tc.tile_pool(name="gp0", bufs=4) as gp0,
                tc.tile_pool(name="gp1", bufs=4) as gp1,
                tc.tile_pool(name="opool", bufs=64) as opool,
                tc.tile_pool(name="pagg", bufs=2, space="PSUM") as pagg,
                tc.tile_pool(name="px1", bufs=2, space="PSUM") as px1,
                tc.tile_pool(name="ptp", bufs=1, space="PSUM") as ptp,
                tc.tile_pool(name="ppr", bufs=1, space="PSUM") as ppr,
                tc.tile_pool(name="dwork", bufs=3) as dwork,
                tc.tile_pool(name="dsmall", bufs=4) as dsmall,
                tc.tile_pool(name="ptpool", bufs=1) as ptpool,
                tc.tile_pool(name="bpp", bufs=1, space="PSUM") as bpp,
                tc.tile_pool(name="bstage", bufs=2) as bstage,
            ):
                lookup = stream_batches((gp0, gp1))
                pt_sb = ptpool.tile([2, cp], F32)
                STG = 7
                stage = None
                for w in range(nw):
                    rows = wchunks[w]
                    aggT = pagg.tile([D, 128], F32, tag="aggT")
                    for k, (g, s) in enumerate(rows):
                        gb, j = lookup[g]
                        o = onehot(opool, g)
                        nc.tensor.matmul(
                            aggT[:], gb[:, j, :], o[:],
                            start=(k == 0), stop=(k == len(rows) - 1),
                        )
                    meanT = dwork.tile([D, 128], F32, tag="meanT")
                    nc.scalar.activation(
                        meanT[:], aggT[:], mybir.ActivationFunctionType.Copy)

                    x1p = px1.tile([128, HID], F32, tag="x1")
                    nc.tensor.matmul(
                        x1p[:], xt_sb[:, 128 * w : 128 * (w + 1)], wcb_sb[:],
                        start=True, stop=False)
                    nc.tensor.matmul(
                        x1p[:], meanT[:], w1l_sb[:], start=False, stop=True)

                    mu = dsmall.tile([128, 1], F32, tag="mu")
                    nc.vector.reduce_sum(
                        out=mu[:], in_=x1p[:], axis=mybir.AxisListType.X)
                    nc.vector.tensor_scalar(
                        out=mu[:], in0=mu[:], scalar1=1.0 / HID,
                        scalar2=None, op0=mybir.AluOpType.mult)
                    xc = dwork.tile([128, HID], F32, tag="xc")
                    nc.vector.tensor_scalar(
                        out=xc[:], in0=x1p[:], scalar1=mu[:], scalar2=None,
                        op0=mybir.AluOpType.subtract)
                    sq = dwork.tile([128, HID], F32, tag="sq")
                    var = dsmall.tile([128, 1], F32, tag="var")
                    nc.scalar.activation(
                        sq[:], xc[:], mybir.ActivationFunctionType.Square,
                        accum_out=var[:])
                    rstd = dsmall.tile([128, 1], F32, tag="rstd")
                    nc.vector.tensor_scalar(
                        out=rstd[:], in0=var[:], scalar1=1.0 / HID,
                        scalar2=LN_EPS, op0=mybir.AluOpType.mult,
                        op1=mybir.AluOpType.add)
                    nc.scalar.activation(
                        rstd[:], rstd[:], mybir.ActivationFunctionType.Sqrt)
                    nc.vector.reciprocal(rstd[:], rstd[:])
                    y = dwork.tile([128, HID], F32, tag="y")
                    nc.vector.tensor_scalar(
                        out=y[:], in0=xc[:], scalar1=rstd[:], scalar2=None,
                        op0=mybir.AluOpType.mult)
                    nc.vector.tensor_tensor(
                        out=y[:], in0=y[:], in1=gamma_sb[:],
                        op=mybir.AluOpType.mult)
                    nc.vector.tensor_tensor(
                        out=y[:], in0=y[:], in1=beta_sb[:],
                        op=mybir.AluOpType.add)
                    m0 = dwork.tile([128, HID], F32, tag="m0")
                    nc.vector.tensor_scalar(
                        out=m0[:], in0=y[:], scalar1=0.0, scalar2=None,
                        op0=mybir.AluOpType.min)
                    ex = dwork.tile([128, HID], F32, tag="ex")
                    nc.scalar.activation(
                        ex[:], m0[:], mybir.ActivationFunctionType.Exp)
                    rm1 = dwork.tile([128, HID], F32, tag="rm1")
                    nc.vector.tensor_scalar(
                        out=rm1[:], in0=y[:], scalar1=0.0, scalar2=-1.0,
                        op0=mybir.AluOpType.max, op1=mybir.AluOpType.add)
                    h = dwork.tile([128, HID], F32, tag="h")
                    nc.vector.tensor_tensor(
                        out=h[:], in0=rm1[:], in1=ex[:],
                        op=mybir.AluOpType.add)

                    prp = ppr.tile([128, 2 * OUT], F32, tag="pr")
                    for hh in range(2):
                        tph = ptp.tile([128, 128], F32, tag="tph")
                        nc.tensor.transpose(
                            tph[:], h[:, 128 * hh : 128 * (hh + 1)],
                            ident_sb[:])
                        hts = dwork.tile([128, 128], F32, tag="hts")
                        nc.scalar.activation(
                            hts[:], tph[:], mybir.ActivationFunctionType.Copy)
                        nc.tensor.matmul(
                            prp[:], hts[:],
                            w2lr_sb[:, 4 * hh : 4 * (hh + 1)],
                            start=(hh == 0), stop=(hh == 1))
                    nc.scalar.activation(
                        pr_sb[:, w, :], prp[:],
                        mybir.ActivationFunctionType.Copy)
                    ptw = ptp.tile([OUT, 128], F32, tag="ptw")
                    nc.tensor.transpose(
                        ptw[:], pr_sb[:, w, 0:OUT], ident_sb[:])
                    nc.scalar.activation(
                        pt_sb[:, 128 * w : 128 * (w + 1)], ptw[:],
                        mybir.ActivationFunctionType.Copy)

                    # local replicated-p block for this window (hidden
                    # under pass-1 gathers); allgathered below
                    pp = bpp.tile([128, D], F32, tag="pb2p")
                    nc.tensor.matmul(
                        pp[:], pt_sb[:, 128 * w : 128 * (w + 1)],
                        i2_sb[:], start=True, stop=True)
                    if w % STG == 0:
                        stage = bstage.tile([128, STG, D], F32, tag="stage")
                    nc.scalar.activation(
                        stage[:, w % STG, :], pp[:],
                        mybir.ActivationFunctionType.Copy)
                    if w % STG == STG - 1:
                        w0 = w - STG + 1
                        nc.sync.dma_start(
                            out=pb2_mine[w0 * 128 : (w0 + STG) * 128, :]
                            .rearrange("(s p) d -> p s d", p=128),
                            in_=stage[:])

            # ============ replicated-p table all-gather ============
            nc.gpsimd.collective_compute(
                "AllGather",
                mybir.AluOpType.bypass,
                replica_groups=[list(range(N_CORES))],
                ins=[pb2_mine.opt()],
                outs=[pb2_full.opt()],
            )

            # ============ pass 2: conv2 agg + output, per window ============
            with (
                tc.tile_pool(name="g2p0", bufs=6) as g2p0,
                tc.tile_pool(name="g2p1", bufs=6) as g2p1,
                tc.tile_pool(name="opool2", bufs=64) as opool2,
                tc.tile_pool(name="pagg2", bufs=2, space="PSUM") as pagg2,
                tc.tile_pool(name="fsmall", bufs=4) as fsmall,
            ):
                lookup2 = gather_batches(
                    (g2p0, g2p1),
                    (pb2_full[0:half, :], pb2_full[half : 2 * half, :]))
                for w in range(nw):
                    rows = wchunks[w]
                    agg2 = pagg2.tile([128, OUT], F32, tag="agg2")
                    for k, (g, s) in enumerate(rows):
                        gb, j = lookup2[g]
                        o = onehot(opool2, g)
                        nc.tensor.matmul(
                            agg2[:], o[:], gb[:, j, 0:OUT],
                            start=(k == 0), stop=(k == len(rows) - 1),
                        )
                    t = fsmall.tile([128, OUT], F32, tag="fo")
                    nc.vector.tensor_tensor(
                        out=t[:], in0=agg2[:], in1=pr_sb[:, w, OUT : 2 * OUT],
                        op=mybir.AluOpType.add)
                    nc.vector.tensor_tensor(
                        out=out_sb[:, w, :], in0=t[:], in1=b2_sb[:],
                        op=mybir.AluOpType.add)
                    if w % 7 == 6:
                        w0 = w - 6
                        nc.sync.dma_start(
                            out=out_d.rearrange(
                                "(w p) c -> p w c", p=128)[:, w0 : w0 + 7, :],
                            in_=out_sb[:, w0 : w0 + 7, :])

    nc.compile()
    return nc


def make_inputs(plan, x, W1l, W1r, b1, Wskip, bskip, gamma, beta, W2l, W2r,
                b2, n_nodes):
    cp, half, npad = plan["cp"], plan["half"], plan["npad"]
    xp = np.zeros((npad, D), np.float32)
    xp[:n_nodes] = np.asarray(x, np.float32)
    wc = np.asarray(W1r, np.float32) + np.asarray(Wskip, np.float32)
    bc = np.asarray(b1, np.float32) + np.asarray(bskip, np.float32)
    wcb = np.concatenate([wc, bc[None, :]], axis=0)
    w2lr_full = np.concatenate(
        [np.asarray(W2l, np.float32), np.asarray(W2r, np.float32)], axis=1)
    w2lr = (w2lr_full.reshape(2, 128, 2 * OUT).transpose(1, 0, 2)
            .reshape(128, 2 * 2 * OUT).copy())
    iota = np.tile(np.arange(128, dtype=np.float32)[None, :], (128, 1))
    ident = np.eye(128, dtype=np.float32)
    i2 = np.zeros((2, D), np.float32)
    i2[0, 0::2] = 1.0
    i2[1, 1::2] = 1.0
    gamma_bc = np.tile(np.asarray(gamma, np.float32)[None, :], (128, 1))
    beta_bc = np.tile(np.asarray(beta, np.float32)[None, :], (128, 1))
    b2_bc = np.tile(np.asarray(b2, np.float32)[None, :], (128, 1))

    common = dict(
        iota=iota, ident=ident, wcb=wcb,
        w1l=np.asarray(W1l, np.float32), w2lr=w2lr,
        gamma_bc=gamma_bc, beta_bc=beta_bc, b2_bc=b2_bc, i2=i2,
    )
    in_maps = []
    for c in range(N_CORES):
        m = dict(common)
        xt = np.ones((D + 1, cp), np.float32)
        xt[:D] = xp[cp * c : cp * (c + 1)].T
        m["xt"] = xt
        m["msg1"] = xp[plan["srcslot"][c]]
        m["gidx"] = plan["gidx_tile"][c]
        m["dstf"] = plan["dstf_tile"][c]
        m["rcf"] = plan["rcf_tile"][c]
        in_maps.append(m)
    return in_maps


_CACHE = {}


def _get_compiled(edge_index, n_nodes):
    key = (edge_index.tobytes()[:512], edge_index.shape, n_nodes)
    if key not in _CACHE:
        plan = make_plan(edge_index, n_nodes)
        nc = build_program(plan)
        _CACHE[key] = (plan, nc)
    return _CACHE[key]


def run(inputs, trace=False):
    x = np.asarray(inputs["x"], np.float32)
    edge_index = np.asarray(inputs["edge_index"], np.int32)
    n_nodes = x.shape[0]
    plan, nc = _get_compiled(edge_index, n_nodes)
    in_maps = make_inputs(
        plan, x, inputs["W1l"], inputs["W1r"], inputs["b1"], inputs["Wskip"],
        inputs["bskip"], inputs["gamma"], inputs["beta"], inputs["W2l"],
        inputs["W2r"], inputs["b2"], n_nodes)
    res = run_bass_kernel_spmd(
        nc, in_maps, list(range(N_CORES)), trace=trace)
    cp = plan["cp"]
    out = np.empty((n_nodes, OUT), np.float32)
    for c in range(N_CORES):
        lo = cp * c
        hi = min(cp * (c + 1), n_nodes)
        out[lo:hi] = res.results[c]["out"][0 : hi - lo]
    return out, res


def kernel(**inputs) -> np.ndarray:
    out, _ = run(inputs)
    return out


# revision 4
# speedup vs baseline: 1.1655x; 1.0035x over previous
"""GeneSAGE v2 on 8 trn2 cores — overlap-first design, f32 numerics.

Structure vs baseline:
- 1/cnt folded into the one-hot build (tensor_scalar is_equal*mult with two
  per-partition AP scalars) -> no count matmuls, no reciprocal chain.
- segment-sum matmul emits the aggregate transposed (gathered rows as
  lhsT, one-hot as rhs) so it lands feature-major (= mean^T with the fold)
  and feeds mean @ W1l directly; x^T+ones row precomputed host-side ->
  no dense-phase transposes of x/mean.
- chunks ordered by window within each stream; the two streams' gather
  batches alternate on Q7 so window w's chunks (both streams) arrive
  early and the dense phase interleaves per window under the gather prep.
- all f32: the harness's rel-err floor (1e-3 denom) demands ~1e-5 abs.
"""

import numpy as np

import concourse.mybir as mybir
from concourse import bacc, bass, tile
from concourse.bass_utils import run_bass_kernel_spmd

F32 = mybir.dt.float32
I16 = mybir.dt.int16

N_CORES = 8
D = 64
HID = 256
OUT = 2
LN_EPS = 1e-5
B_CH = 16
SINGLE_PACKET = False


def make_plan(edge_index: np.ndarray, n_nodes: int):
    cp = int(np.ceil(n_nodes / (N_CORES * 128))) * 128
    nw = cp // 128
    npad = N_CORES * cp
    half = npad // 2
    assert half <= 32768

    src = edge_index[0].astype(np.int64)
    dst = edge_index[1].astype(np.int64)
    E = src.shape[0]

    core = dst // cp
    stream = (src >= half).astype(np.int64)
    win = (dst % cp) // 128
    ngrp = 2 * nw
    key = (core * 2 + stream) * nw + win
    order = np.argsort(key, kind="stable")
    counts = np.bincount(key, minlength=N_CORES * ngrp).reshape(
        N_CORES, 2, nw)
    nchunks = -(-counts.max(axis=0) // 128)  # [2, nw]
    off = np.zeros((2, nw), np.int64)
    running = 0
    for s in range(2):
        for w in range(nw):
            off[s, w] = running
            running += nchunks[s, w]
    c_total = int(running)
    c_lo = int(nchunks[0].sum())
    e_slots = c_total * 128

    sk = key[order]
    grp_start = np.searchsorted(sk, np.arange(N_CORES * ngrp))
    rank = np.arange(E) - grp_start[sk]
    s_of = (sk // nw) % 2
    w_of = sk % nw
    c_of = sk // ngrp
    slot = off[s_of, w_of] * 128 + rank

    cnt = np.bincount(dst, minlength=npad).astype(np.float32)
    rcnt = 1.0 / np.maximum(cnt, 1.0)

    gidx = np.zeros((N_CORES, e_slots), np.int16)
    dstf = np.full((N_CORES, e_slots), -1.0, np.float32)
    rcf = np.ones((N_CORES, e_slots), np.float32)
    srcslot = np.zeros((N_CORES, e_slots), np.int32)
    gidx[c_of, slot] = (src[order] - s_of * half).astype(np.int16)
    dstf[c_of, slot] = (dst[order] % 128).astype(np.float32)
    rcf[c_of, slot] = rcnt[dst[order]]
    srcslot[c_of, slot] = src[order].astype(np.int32)

    a = gidx.reshape(N_CORES, e_slots // 16, 16).transpose(0, 2, 1)
    gidx_tile = np.tile(a, (1, 8, 1)).copy()  # [c, 128, J]
    dstf_tile = np.ascontiguousarray(
        dstf.reshape(N_CORES, c_total, 128).transpose(0, 2, 1))
    rcf_tile = np.ascontiguousarray(
        rcf.reshape(N_CORES, c_total, 128).transpose(0, 2, 1))

    # per window: ordered chunk list [(chunk_id, stream), ...]
    wchunks = []
    for w in range(nw):
        rows = []
        for s in range(2):
            f = int(off[s, w])
            for g in range(f, f + int(nchunks[s, w])):
                rows.append((g, s))
        wchunks.append(rows)

    return dict(
        cp=cp, nw=nw, npad=npad, half=half, c_total=c_total, c_lo=c_lo,
        wchunks=wchunks, gidx_tile=gidx_tile, dstf_tile=dstf_tile,
        rcf_tile=rcf_tile, srcslot=srcslot, e_slots=e_slots,
    )


def build_program(plan):
    cp, nw, half = plan["cp"], plan["nw"], plan["half"]
    c_total, c_lo = plan["c_total"], plan["c_lo"]
    wchunks = plan["wchunks"]
    J = c_total * 8

    nc = bacc.Bacc("TRN2", target_bir_lowering=False, debug=False,
                   num_devices=N_CORES, num_swdge_queues=2)

    def inp(name, shape, dt=F32):
        return nc.dram_tensor(name, shape, dt, kind="ExternalInput").ap()

    msg1_d = inp("msg1", [c_total * 128, D])
    gidx_d = inp("gidx", [128, J], I16)
    dstf_d = inp("dstf", [128, c_total])
    rcf_d = inp("rcf", [128, c_total])
    iota_d = inp("iota", [128, 128])
    ident_d = inp("ident", [128, 128])
    xt_d = inp("xt", [D + 1, cp])
    wcb_d = inp("wcb", [D + 1, HID])
    w1l_d = inp("w1l", [D, HID])
    w2lr_d = inp("w2lr", [128, 2 * 2 * OUT])
    gamma_d = inp("gamma_bc", [128, HID])
    beta_d = inp("beta_bc", [128, HID])
    b2_d = inp("b2_bc", [128, OUT])
    i2_d = inp("i2", [2, D])
    out_d = nc.dram_tensor("out", [cp, OUT], F32, kind="ExternalOutput").ap()

    ranges = [(0, c_lo), (c_lo, c_total)]

    with tile.TileContext(nc) as tc:
        with (
            tc.tile_pool(name="res", bufs=1) as res,
            tc.tile_pool(name="dram", bufs=1, space="DRAM") as dram,
        ):
            def load(name, ap, shape, dt=F32):
                t = res.tile(shape, dt, tag=name, name=name)
                nc.sync.dma_start(out=t[:], in_=ap)
                return t

            gidx_sb = load("gidx", gidx_d, [128, J], I16)
            dstf_sb = load("dstf", dstf_d, [128, c_total])
            rcf_sb = load("rcf", rcf_d, [128, c_total])
            iota_sb = load("iota", iota_d, [128, 128])
            ident_sb = load("ident", ident_d, [128, 128])
            xt_sb = load("xt", xt_d, [D + 1, cp])
            wcb_sb = load("wcb", wcb_d, [D + 1, HID])
            w1l_sb = load("w1l", w1l_d, [D, HID])
            w2lr_sb = load("w2lr", w2lr_d, [128, 2 * 2 * OUT])
            gamma_sb = load("gamma", gamma_d, [128, HID])
            beta_sb = load("beta", beta_d, [128, HID])
            b2_sb = load("b2", b2_d, [128, OUT])
            i2_sb = load("i2", i2_d, [2, D])

            pr_sb = res.tile([128, nw, 2 * OUT], F32, tag="prsb", name="prsb")
            out_sb = res.tile([128, nw, OUT], F32, tag="outsb", name="outsb")

            pb2_mine = dram.tile([cp, D], F32)
            pb2_full = dram.tile([N_CORES * cp, D], F32)

            def onehot(opool, g):
                o = opool.tile([128, 128], F32, tag="O")
                nc.vector.tensor_scalar(
                    out=o[:], in0=iota_sb[:],
                    scalar1=dstf_sb[:, g : g + 1],
                    scalar2=rcf_sb[:, g : g + 1],
                    op0=mybir.AluOpType.is_equal,
                    op1=mybir.AluOpType.mult,
                )
                return o

            def batch_ranges():
                per_stream = []
                for s in range(2):
                    lo, hi = ranges[s]
                    per_stream.append(
                        [(b0, min(b0 + B_CH, hi))
                         for b0 in range(lo, hi, B_CH)])
                out = []
                nb = max(len(per_stream[0]), len(per_stream[1]))
                for k in range(nb):
                    for s in range(2):
                        if k < len(per_stream[s]):
                            out.append((s, per_stream[s][k]))
                return out

            def stream_batches(pools):
                """Pass 1: host pre-permuted messages, plain strided DMA."""
                lookup = {}
                for s, (b0, b1) in batch_ranges():
                    g = pools[s].tile([128, B_CH, D], F32, tag="gbuf")
                    nc.sync.dma_start(
                        out=g[:, 0 : b1 - b0, :],
                        in_=msg1_d[b0 * 128 : b1 * 128, :].rearrange(
                            "(c p) d -> p c d", p=128))
                    for gg in range(b0, b1):
                        lookup[gg] = (g, gg - b0)
                return lookup

            def gather_batches(pools, tables):
                """Pass 2: Q7 dma_gather, alternating the two streams."""
                lookup = {}
                for s, (b0, b1) in batch_ranges():
                    g = pools[s].tile([128, B_CH, D], F32, tag="gbuf")
                    n_idx = (b1 - b0) * 128
                    nc.gpsimd.dma_gather(
                        out_ap=g[:, 0 : b1 - b0, :],
                        in_ap=tables[s],
                        idxs_ap=gidx_sb[:, b0 * 8 : b1 * 8],
                        num_idxs=n_idx,
                        num_idxs_reg=n_idx,
                        elem_size=D,
                        single_packet=SINGLE_PACKET,
                        queue_num=s,
                    )
                    for gg in range(b0, b1):
                        lookup[gg] = (g, gg - b0)
                return lookup

            # ============ pass 1: conv1 agg + dense, per window ============
            with (
                tc.tile_pool(name="gp0", bufs=4) as gp0,
                tc.tile_pool(name="gp1", bufs=4) as gp1,
                tc.tile_pool(name="opool", bufs=64) as opool,
                tc.tile_pool(name="pagg", bufs=2, space="PSUM") as pagg,
                tc.tile_pool(name="px1", bufs=2, space="PSUM") as px1,
                tc.tile_pool(name="ptp", bufs=1, space="PSUM") as ptp,
                tc.tile_pool(name="ppr", bufs=1, space="PSUM") as ppr,
                tc.tile_pool(name="dwork", bufs=3) as dwork,
                tc.tile_pool(name="dsmall", bufs=4) as dsmall,
                tc.tile_pool(name="ptpool", bufs=1) as ptpool,
                tc.tile_pool(name="bpp", bufs=1, space="PSUM") as bpp,
                tc.tile_pool(name="bstage", bufs=2) as bstage,
            ):
                lookup = stream_batches((gp0, gp1))
                pt_sb = ptpool.tile([2, cp], F32)
                STG = 7
                stage = None
                ohots = {g: onehot(opool, g) for (g, s) in wchunks[0]}
                for w in range(nw):
                    rows = wchunks[w]
                    cur = ohots
                    if w + 1 < nw:
                        ohots = {g: onehot(opool, g)
                                 for (g, s) in wchunks[w + 1]}
                    aggT = pagg.tile([D, 128], F32, tag="aggT")
                    for k, (g, s) in enumerate(rows):
                        gb, j = lookup[g]
                        o = cur[g]
                        nc.tensor.matmul(
                            aggT[:], gb[:, j, :], o[:],
                            start=(k == 0), stop=(k == len(rows) - 1),
                        )
                    meanT = dwork.tile([D, 128], F32, tag="meanT")
                    nc.scalar.activation(
                        meanT[:], aggT[:], mybir.ActivationFunctionType.Copy)

                    x1p = px1.tile([128, HID], F32, tag="x1")
                    nc.tensor.matmul(
                        x1p[:], xt_sb[:, 128 * w : 128 * (w + 1)], wcb_sb[:],
                        start=True, stop=False)
                    nc.tensor.matmul(
                        x1p[:], meanT[:], w1l_sb[:], start=False, stop=True)

                    mu = dsmall.tile([128, 1], F32, tag="mu")
                    nc.vector.reduce_sum(
                        out=mu[:], in_=x1p[:], axis=mybir.AxisListType.X)
                    nc.vector.tensor_scalar(
                        out=mu[:], in0=mu[:], scalar1=1.0 / HID,
                        scalar2=None, op0=mybir.AluOpType.mult)
                    xc = dwork.tile([128, HID], F32, tag="xc")
                    nc.vector.tensor_scalar(
                        out=xc[:], in0=x1p[:], scalar1=mu[:], scalar2=None,
                        op0=mybir.AluOpType.subtract)
                    sq = dwork.tile([128, HID], F32, tag="sq")
                    var = dsmall.tile([128, 1], F32, tag="var")
                    nc.scalar.activation(
                        sq[:], xc[:], mybir.ActivationFunctionType.Square,
                        accum_out=var[:])
                    rstd = dsmall.tile([128, 1], F32, tag="rstd")
                    nc.vector.tensor_scalar(
                        out=rstd[:], in0=var[:], scalar1=1.0 / HID,
                        scalar2=LN_EPS, op0=mybir.AluOpType.mult,
                        op1=mybir.AluOpType.add)
                    nc.scalar.activation(
                        rstd[:], rstd[:], mybir.ActivationFunctionType.Sqrt)
                    nc.vector.reciprocal(rstd[:], rstd[:])
                    y = dwork.tile([128, HID], F32, tag="y")
                    nc.vector.tensor_scalar(
                        out=y[:], in0=xc[:], scalar1=rstd[:], scalar2=None,
                        op0=mybir.AluOpType.mult)
                    nc.vector.tensor_tensor(
                        out=y[:], in0=y[:], in1=gamma_sb[:],
                        op=mybir.AluOpType.mult)
                    nc.vector.tensor_tensor(
                        out=y[:], in0=y[:], in1=beta_sb[:],
                        op=mybir.AluOpType.add)
                    m0 = dwork.tile([128, HID], F32, tag="m0")
                    nc.vector.tensor_scalar(
                        out=m0[:], in0=y[:], scalar1=0.0, scalar2=None,
                        op0=mybir.AluOpType.min)
                    ex = dwork.tile([128, HID], F32, tag="ex")
                    nc.scalar.activation(
                        ex[:], m0[:], mybir.ActivationFunctionType.Exp)
                    rm1 = dwork.tile([128, HID], F32, tag="rm1")
                    nc.vector.tensor_scalar(
                        out=rm1[:], in0=y[:], scalar1=0.0, scalar2=-1.0,
                        op0=mybir.AluOpType.max, op1=mybir.AluOpType.add)
                    h = dwork.tile([128, HID], F32, tag="h")
                    nc.vector.tensor_tensor(
                        out=h[:], in0=rm1[:], in1=ex[:],
                        op=mybir.AluOpType.add)

                    prp = ppr.tile([128, 2 * OUT], F32, tag="pr")
                    for hh in range(2):
                        tph = ptp.tile([128, 128], F32, tag="tph")
                        nc.tensor.transpose(
                            tph[:], h[:, 128 * hh : 128 * (hh + 1)],
                            ident_sb[:])
                        hts = dwork.tile([128, 128], F32, tag="hts")
                        nc.scalar.activation(
                            hts[:], tph[:], mybir.ActivationFunctionType.Copy)
                        nc.tensor.matmul(
                            prp[:], hts[:],
                            w2lr_sb[:, 4 * hh : 4 * (hh + 1)],
                            start=(hh == 0), stop=(hh == 1))
                    nc.scalar.activation(
                        pr_sb[:, w, :], prp[:],
                        mybir.ActivationFunctionType.Copy)
                    ptw = ptp.tile([OUT, 128], F32, tag="ptw")
                    nc.tensor.transpose(
                        ptw[:], pr_sb[:, w, 0:OUT], ident_sb[:])
                    nc.scalar.activation(
                        pt_sb[:, 128 * w : 128 * (w + 1)], ptw[:],
                        mybir.ActivationFunctionType.Copy)

                    # local replicated-p block for this window (hidden
                    # under pass-1 gathers); allgathered below
                    pp = bpp.tile([128, D], F32, tag="pb2p")
                    nc.tensor.matmul(
                        pp[:], pt_sb[:, 128 * w : 128 * (w + 1)],
                        i2_sb[:], start=True, stop=True)
                    if w % STG == 0:
                        stage = bstage.tile([128, STG, D], F32, tag="stage")
                    nc.scalar.activation(
                        stage[:, w % STG, :], pp[:],
                        mybir.ActivationFunctionType.Copy)
                    if w % STG == STG - 1:
                        w0 = w - STG + 1
                        nc.sync.dma_start(
                            out=pb2_mine[w0 * 128 : (w0 + STG) * 128, :]
                            .rearrange("(s p) d -> p s d", p=128),
                            in_=stage[:])

            # ============ replicated-p table all-gather ============
            nc.gpsimd.collective_compute(
                "AllGather",
                mybir.AluOpType.bypass,
                replica_groups=[list(range(N_CORES))],
                ins=[pb2_mine.opt()],
                outs=[pb2_full.opt()],
            )

            # ============ pass 2: conv2 agg + output, per window ============
            with (
                tc.tile_pool(name="g2p0", bufs=6) as g2p0,
                tc.tile_pool(name="g2p1", bufs=6) as g2p1,
                tc.tile_pool(name="opool2", bufs=64) as opool2,
                tc.tile_pool(name="pagg2", bufs=2, space="PSUM") as pagg2,
                tc.tile_pool(name="fsmall", bufs=4) as fsmall,
            ):
                lookup2 = gather_batches(
                    (g2p0, g2p1),
                    (pb2_full[0:half, :], pb2_full[half : 2 * half, :]))
                for w in range(nw):
                    rows = wchunks[w]
                    agg2 = pagg2.tile([128, OUT], F32, tag="agg2")
                    for k, (g, s) in enumerate(rows):
                        gb, j = lookup2[g]
                        o = onehot(opool2, g)
                        nc.tensor.matmul(
                            agg2[:], o[:], gb[:, j, 0:OUT],
                            start=(k == 0), stop=(k == len(rows) - 1),
                        )
                    t = fsmall.tile([128, OUT], F32, tag="fo")
                    nc.vector.tensor_tensor(
                        out=t[:], in0=agg2[:], in1=pr_sb[:, w, OUT : 2 * OUT],
                        op=mybir.AluOpType.add)
                    nc.vector.tensor_tensor(
                        out=out_sb[:, w, :], in0=t[:], in1=b2_sb[:],
                        op=mybir.AluOpType.add)
                    if w % 7 == 6:
                        w0 = w - 6
                        nc.sync.dma_start(
                            out=out_d.rearrange(
                                "(w p) c -> p w c", p=128)[:, w0 : w0 + 7, :],
                            in_=out_sb[:, w0 : w0 + 7, :])

    nc.compile()
    return nc


def make_inputs(plan, x, W1l, W1r, b1, Wskip, bskip, gamma, beta, W2l, W2r,
                b2, n_nodes):
    cp, half, npad = plan["cp"], plan["half"], plan["npad"]
    xp = np.zeros((npad, D), np.float32)
    xp[:n_nodes] = np.asarray(x, np.float32)
    wc = np.asarray(W1r, np.float32) + np.asarray(Wskip, np.float32)
    bc = np.asarray(b1, np.float32) + np.asarray(bskip, np.float32)
    wcb = np.concatenate([wc, bc[None, :]], axis=0)
    w2lr_full = np.concatenate(
        [np.asarray(W2l, np.float32), np.asarray(W2r, np.float32)], axis=1)
    w2lr = (w2lr_full.reshape(2, 128, 2 * OUT).transpose(1, 0, 2)
            .reshape(128, 2 * 2 * OUT).copy())
    iota = np.tile(np.arange(128, dtype=np.float32)[None, :], (128, 1))
    ident = np.eye(128, dtype=np.float32)
    i2 = np.zeros((2, D), np.float32)
    i2[0, 0::2] = 1.0
    i2[1, 1::2] = 1.0
    gamma_bc = np.tile(np.asarray(gamma, np.float32)[None, :], (128, 1))
    beta_bc = np.tile(np.asarray(beta, np.float32)[None, :], (128, 1))
    b2_bc = np.tile(np.asarray(b2, np.float32)[None, :], (128, 1))

    common = dict(
        iota=iota, ident=ident, wcb=wcb,
        w1l=np.asarray(W1l, np.float32), w2lr=w2lr,
        gamma_bc=gamma_bc, beta_bc=beta_bc, b2_bc=b2_bc, i2=i2,
    )
    in_maps = []
    for c in range(N_CORES):
        m = dict(common)
        xt = np.ones((D + 1, cp), np.float32)
        xt[:D] = xp[cp * c : cp * (c + 1)].T
        m["xt"] = xt
        m["msg1"] = xp[plan["srcslot"][c]]
        m["gidx"] = plan["gidx_tile"][c]
        m["dstf"] = plan["dstf_tile"][c]
        m["rcf"] = plan["rcf_tile"][c]
        in_maps.append(m)
    return in_maps


_CACHE = {}


def _get_compiled(edge_index, n_nodes):
    key = (edge_index.tobytes()[:512], edge_index.shape, n_nodes)
    if key not in _CACHE:
        plan = make_plan(edge_index, n_nodes)
        nc = build_program(plan)
        _CACHE[key] = (plan, nc)
    return _CACHE[key]


def run(inputs, trace=False):
    x = np.asarray(inputs["x"], np.float32)
    edge_index = np.asarray(inputs["edge_index"], np.int32)
    n_nodes = x.shape[0]
    plan, nc = _get_compiled(edge_index, n_nodes)
    in_maps = make_inputs(
        plan, x, inputs["W1l"], inputs["W1r"], inputs["b1"], inputs["Wskip"],
        inputs["bskip"], inputs["gamma"], inputs["beta"], inputs["W2l"],
        inputs["W2r"], inputs["b2"], n_nodes)
    res = run_bass_kernel_spmd(
        nc, in_maps, list(range(N_CORES)), trace=trace)
    cp = plan["cp"]
    out = np.empty((n_nodes, OUT), np.float32)
    for c in range(N_CORES):
        lo = cp * c
        hi = min(cp * (c + 1), n_nodes)
        out[lo:hi] = res.results[c]["out"][0 : hi - lo]
    return out, res


def kernel(**inputs) -> np.ndarray:
    out, _ = run(inputs)
    return out


# revision 6
# speedup vs baseline: 1.2022x; 1.0315x over previous
"""GeneSAGE v2 on 8 trn2 cores — overlap-first design, f32 numerics.

Structure vs baseline:
- 1/cnt folded into the one-hot build (tensor_scalar is_equal*mult with two
  per-partition AP scalars) -> no count matmuls, no reciprocal chain.
- segment-sum matmul emits the aggregate transposed (gathered rows as
  lhsT, one-hot as rhs) so it lands feature-major (= mean^T with the fold)
  and feeds mean @ W1l directly; x^T+ones row precomputed host-side ->
  no dense-phase transposes of x/mean.
- chunks ordered by window within each stream; the two streams' gather
  batches alternate on Q7 so window w's chunks (both streams) arrive
  early and the dense phase interleaves per window under the gather prep.
- all f32: the harness's rel-err floor (1e-3 denom) demands ~1e-5 abs.
"""

import numpy as np

import concourse.mybir as mybir
from concourse import bacc, bass, tile
from concourse.bass_utils import run_bass_kernel_spmd

F32 = mybir.dt.float32
I16 = mybir.dt.int16

N_CORES = 8
D = 64
HID = 256
OUT = 2
LN_EPS = 1e-5
B_CH = 16
SINGLE_PACKET = False


def make_plan(edge_index: np.ndarray, n_nodes: int):
    cp = int(np.ceil(n_nodes / (N_CORES * 128))) * 128
    nw = cp // 128
    npad = N_CORES * cp
    half = npad // 2
    assert half <= 32768

    src = edge_index[0].astype(np.int64)
    dst = edge_index[1].astype(np.int64)
    E = src.shape[0]

    core = dst // cp
    stream = (src >= half).astype(np.int64)
    win = (dst % cp) // 128
    ngrp = 2 * nw
    key = (core * 2 + stream) * nw + win
    order = np.argsort(key, kind="stable")
    counts = np.bincount(key, minlength=N_CORES * ngrp).reshape(
        N_CORES, 2, nw)
    nchunks = -(-counts.max(axis=0) // 128)  # [2, nw]
    off = np.zeros((2, nw), np.int64)
    running = 0
    for s in range(2):
        for w in range(nw):
            off[s, w] = running
            running += nchunks[s, w]
    c_total = int(running)
    c_lo = int(nchunks[0].sum())
    e_slots = c_total * 128

    sk = key[order]
    grp_start = np.searchsorted(sk, np.arange(N_CORES * ngrp))
    rank = np.arange(E) - grp_start[sk]
    s_of = (sk // nw) % 2
    w_of = sk % nw
    c_of = sk // ngrp
    slot = off[s_of, w_of] * 128 + rank

    cnt = np.bincount(dst, minlength=npad).astype(np.float32)
    rcnt = 1.0 / np.maximum(cnt, 1.0)

    gidx = np.zeros((N_CORES, e_slots), np.int16)
    dstf = np.full((N_CORES, e_slots), -1.0, np.float32)
    rcf = np.ones((N_CORES, e_slots), np.float32)
    srcslot = np.zeros((N_CORES, e_slots), np.int32)
    gidx[c_of, slot] = (src[order] - s_of * half).astype(np.int16)
    dstf[c_of, slot] = (dst[order] % 128).astype(np.float32)
    rcf[c_of, slot] = rcnt[dst[order]]
    srcslot[c_of, slot] = src[order].astype(np.int32)

    a = gidx.reshape(N_CORES, e_slots // 16, 16).transpose(0, 2, 1)
    gidx_tile = np.tile(a, (1, 8, 1)).copy()  # [c, 128, J]
    dstf_tile = np.ascontiguousarray(
        dstf.reshape(N_CORES, c_total, 128).transpose(0, 2, 1))
    rcf_tile = np.ascontiguousarray(
        rcf.reshape(N_CORES, c_total, 128).transpose(0, 2, 1))

    # per window: ordered chunk list [(chunk_id, stream), ...]
    wchunks = []
    for w in range(nw):
        rows = []
        for s in range(2):
            f = int(off[s, w])
            for g in range(f, f + int(nchunks[s, w])):
                rows.append((g, s))
        wchunks.append(rows)

    return dict(
        cp=cp, nw=nw, npad=npad, half=half, c_total=c_total, c_lo=c_lo,
        wchunks=wchunks, gidx_tile=gidx_tile, dstf_tile=dstf_tile,
        rcf_tile=rcf_tile, srcslot=srcslot, e_slots=e_slots,
    )


def build_program(plan):
    cp, nw, half = plan["cp"], plan["nw"], plan["half"]
    c_total, c_lo = plan["c_total"], plan["c_lo"]
    wchunks = plan["wchunks"]
    J = c_total * 8

    nc = bacc.Bacc("TRN2", target_bir_lowering=False, debug=False,
                   num_devices=N_CORES, num_swdge_queues=4)

    def inp(name, shape, dt=F32):
        return nc.dram_tensor(name, shape, dt, kind="ExternalInput").ap()

    msg1_d = inp("msg1", [c_total * 128, D])
    gidx_d = inp("gidx", [128, J], I16)
    dstf_d = inp("dstf", [128, c_total])
    rcf_d = inp("rcf", [128, c_total])
    iota_d = inp("iota", [128, 128])
    ident_d = inp("ident", [128, 128])
    xt_d = inp("xt", [D + 1, cp])
    wcb_d = inp("wcb", [D + 1, HID])
    w1l_d = inp("w1l", [D, HID])
    w2lr_d = inp("w2lr", [128, 2 * 2 * OUT])
    gamma_d = inp("gamma_bc", [128, HID])
    beta_d = inp("beta_bc", [128, HID])
    b2_d = inp("b2_bc", [128, OUT])
    i2_d = inp("i2", [2, D])
    out_d = nc.dram_tensor("out", [cp, OUT], F32, kind="ExternalOutput").ap()

    ranges = [(0, c_lo), (c_lo, c_total)]

    with tile.TileContext(nc) as tc:
        with (
            tc.tile_pool(name="res", bufs=1) as res,
            tc.tile_pool(name="dram", bufs=1, space="DRAM") as dram,
        ):
            def load(name, ap, shape, dt=F32):
                t = res.tile(shape, dt, tag=name, name=name)
                nc.sync.dma_start(out=t[:], in_=ap)
                return t

            gidx_sb = load("gidx", gidx_d, [128, J], I16)
            dstf_sb = load("dstf", dstf_d, [128, c_total])
            rcf_sb = load("rcf", rcf_d, [128, c_total])
            iota_sb = load("iota", iota_d, [128, 128])
            ident_sb = load("ident", ident_d, [128, 128])
            xt_sb = load("xt", xt_d, [D + 1, cp])
            wcb_sb = load("wcb", wcb_d, [D + 1, HID])
            w1l_sb = load("w1l", w1l_d, [D, HID])
            w2lr_sb = load("w2lr", w2lr_d, [128, 2 * 2 * OUT])
            gamma_sb = load("gamma", gamma_d, [128, HID])
            beta_sb = load("beta", beta_d, [128, HID])
            b2_sb = load("b2", b2_d, [128, OUT])
            i2_sb = load("i2", i2_d, [2, D])

            pr_sb = res.tile([128, nw, 2 * OUT], F32, tag="prsb", name="prsb")
            out_sb = res.tile([128, nw, OUT], F32, tag="outsb", name="outsb")

            pb2_mine = dram.tile([cp, D], F32)
            pb2_full = dram.tile([N_CORES * cp, D], F32)

            def onehot(opool, g):
                o = opool.tile([128, 128], F32, tag="O")
                nc.vector.tensor_scalar(
                    out=o[:], in0=iota_sb[:],
                    scalar1=dstf_sb[:, g : g + 1],
                    scalar2=rcf_sb[:, g : g + 1],
                    op0=mybir.AluOpType.is_equal,
                    op1=mybir.AluOpType.mult,
                )
                return o

            def batch_ranges():
                per_stream = []
                for s in range(2):
                    lo, hi = ranges[s]
                    per_stream.append(
                        [(b0, min(b0 + B_CH, hi))
                         for b0 in range(lo, hi, B_CH)])
                out = []
                nb = max(len(per_stream[0]), len(per_stream[1]))
                for k in range(nb):
                    for s in range(2):
                        if k < len(per_stream[s]):
                            out.append((s, per_stream[s][k]))
                return out

            def stream_batches(pools):
                """Pass 1: host pre-permuted messages, plain strided DMA."""
                lookup = {}
                for s, (b0, b1) in batch_ranges():
                    g = pools[s].tile([128, B_CH, D], F32, tag="gbuf")
                    nc.sync.dma_start(
                        out=g[:, 0 : b1 - b0, :],
                        in_=msg1_d[b0 * 128 : b1 * 128, :].rearrange(
                            "(c p) d -> p c d", p=128))
                    for gg in range(b0, b1):
                        lookup[gg] = (g, gg - b0)
                return lookup

            def gather_batches(pools, tables):
                """Pass 2: Q7 dma_gather, alternating the two streams."""
                lookup = {}
                for ci, (s, (b0, b1)) in enumerate(batch_ranges()):
                    g = pools[s].tile([128, B_CH, D], F32, tag="gbuf")
                    n_idx = (b1 - b0) * 128
                    nc.gpsimd.dma_gather(
                        out_ap=g[:, 0 : b1 - b0, :],
                        in_ap=tables[s],
                        idxs_ap=gidx_sb[:, b0 * 8 : b1 * 8],
                        num_idxs=n_idx,
                        num_idxs_reg=n_idx,
                        elem_size=D,
                        single_packet=SINGLE_PACKET,
                        queue_num=ci % 4,
                    )
                    for gg in range(b0, b1):
                        lookup[gg] = (g, gg - b0)
                return lookup

            # ============ pass 1: conv1 agg + dense, per window ============
            with (
                tc.tile_pool(name="gp0", bufs=5) as gp0,
                tc.tile_pool(name="gp1", bufs=5) as gp1,
                tc.tile_pool(name="opool", bufs=64) as opool,
                tc.tile_pool(name="pagg", bufs=2, space="PSUM") as pagg,
                tc.tile_pool(name="px1", bufs=2, space="PSUM") as px1,
                tc.tile_pool(name="ptp", bufs=1, space="PSUM") as ptp,
                tc.tile_pool(name="ppr", bufs=1, space="PSUM") as ppr,
                tc.tile_pool(name="dwork", bufs=3) as dwork,
                tc.tile_pool(name="dsmall", bufs=4) as dsmall,
                tc.tile_pool(name="ptpool", bufs=1) as ptpool,
                tc.tile_pool(name="bpp", bufs=1, space="PSUM") as bpp,
                tc.tile_pool(name="bstage", bufs=2) as bstage,
            ):
                lookup = stream_batches((gp0, gp1))
                pt_sb = ptpool.tile([2, cp], F32)
                STG = 7
                stage = None
                ohots = {g: onehot(opool, g) for (g, s) in wchunks[0]}
                for w in range(nw):
                    rows = wchunks[w]
                    cur = ohots
                    if w + 1 < nw:
                        ohots = {g: onehot(opool, g)
                                 for (g, s) in wchunks[w + 1]}
                    aggT = pagg.tile([D, 128], F32, tag="aggT")
                    for k, (g, s) in enumerate(rows):
                        gb, j = lookup[g]
                        o = cur[g]
                        nc.tensor.matmul(
                            aggT[:], gb[:, j, :], o[:],
                            start=(k == 0), stop=(k == len(rows) - 1),
                        )
                    meanT = dwork.tile([D, 128], F32, tag="meanT")
                    nc.scalar.activation(
                        meanT[:], aggT[:], mybir.ActivationFunctionType.Copy)

                    x1p = px1.tile([128, HID], F32, tag="x1")
                    nc.tensor.matmul(
                        x1p[:], xt_sb[:, 128 * w : 128 * (w + 1)], wcb_sb[:],
                        start=True, stop=False)
                    nc.tensor.matmul(
                        x1p[:], meanT[:], w1l_sb[:], start=False, stop=True)

                    mu = dsmall.tile([128, 1], F32, tag="mu")
                    nc.vector.reduce_sum(
                        out=mu[:], in_=x1p[:], axis=mybir.AxisListType.X)
                    nc.vector.tensor_scalar(
                        out=mu[:], in0=mu[:], scalar1=1.0 / HID,
                        scalar2=None, op0=mybir.AluOpType.mult)
                    xc = dwork.tile([128, HID], F32, tag="xc")
                    nc.vector.tensor_scalar(
                        out=xc[:], in0=x1p[:], scalar1=mu[:], scalar2=None,
                        op0=mybir.AluOpType.subtract)
                    sq = dwork.tile([128, HID], F32, tag="sq")
                    var = dsmall.tile([128, 1], F32, tag="var")
                    nc.scalar.activation(
                        sq[:], xc[:], mybir.ActivationFunctionType.Square,
                        accum_out=var[:])
                    rstd = dsmall.tile([128, 1], F32, tag="rstd")
                    nc.vector.tensor_scalar(
                        out=rstd[:], in0=var[:], scalar1=1.0 / HID,
                        scalar2=LN_EPS, op0=mybir.AluOpType.mult,
                        op1=mybir.AluOpType.add)
                    nc.scalar.activation(
                        rstd[:], rstd[:], mybir.ActivationFunctionType.Sqrt)
                    nc.vector.reciprocal(rstd[:], rstd[:])
                    y = dwork.tile([128, HID], F32, tag="y")
                    nc.vector.tensor_scalar(
                        out=y[:], in0=xc[:], scalar1=rstd[:], scalar2=None,
                        op0=mybir.AluOpType.mult)
                    nc.vector.tensor_tensor(
                        out=y[:], in0=y[:], in1=gamma_sb[:],
                        op=mybir.AluOpType.mult)
                    nc.vector.tensor_tensor(
                        out=y[:], in0=y[:], in1=beta_sb[:],
                        op=mybir.AluOpType.add)
                    m0 = dwork.tile([128, HID], F32, tag="m0")
                    nc.vector.tensor_scalar(
                        out=m0[:], in0=y[:], scalar1=0.0, scalar2=None,
                        op0=mybir.AluOpType.min)
                    ex = dwork.tile([128, HID], F32, tag="ex")
                    nc.scalar.activation(
                        ex[:], m0[:], mybir.ActivationFunctionType.Exp)
                    rm1 = dwork.tile([128, HID], F32, tag="rm1")
                    nc.vector.tensor_scalar(
                        out=rm1[:], in0=y[:], scalar1=0.0, scalar2=-1.0,
                        op0=mybir.AluOpType.max, op1=mybir.AluOpType.add)
                    h = dwork.tile([128, HID], F32, tag="h")
                    nc.vector.tensor_tensor(
                        out=h[:], in0=rm1[:], in1=ex[:],
                        op=mybir.AluOpType.add)

                    prp = ppr.tile([128, 2 * OUT], F32, tag="pr")
                    for hh in range(2):
                        tph = ptp.tile([128, 128], F32, tag="tph")
                        nc.tensor.transpose(
                            tph[:], h[:, 128 * hh : 128 * (hh + 1)],
                            ident_sb[:])
                        hts = dwork.tile([128, 128], F32, tag="hts")
                        nc.scalar.activation(
                            hts[:], tph[:], mybir.ActivationFunctionType.Copy)
                        nc.tensor.matmul(
                            prp[:], hts[:],
                            w2lr_sb[:, 4 * hh : 4 * (hh + 1)],
                            start=(hh == 0), stop=(hh == 1))
                    nc.scalar.activation(
                        pr_sb[:, w, :], prp[:],
                        mybir.ActivationFunctionType.Copy)
                    ptw = ptp.tile([OUT, 128], F32, tag="ptw")
                    nc.tensor.transpose(
                        ptw[:], pr_sb[:, w, 0:OUT], ident_sb[:])
                    nc.scalar.activation(
                        pt_sb[:, 128 * w : 128 * (w + 1)], ptw[:],
                        mybir.ActivationFunctionType.Copy)

                    # local replicated-p block for this window (hidden
                    # under pass-1 gathers); allgathered below
                    pp = bpp.tile([128, D], F32, tag="pb2p")
                    nc.tensor.matmul(
                        pp[:], pt_sb[:, 128 * w : 128 * (w + 1)],
                        i2_sb[:], start=True, stop=True)
                    if w % STG == 0:
                        stage = bstage.tile([128, STG, D], F32, tag="stage")
                    nc.scalar.activation(
                        stage[:, w % STG, :], pp[:],
                        mybir.ActivationFunctionType.Copy)
                    if w % STG == STG - 1:
                        w0 = w - STG + 1
                        nc.sync.dma_start(
                            out=pb2_mine[w0 * 128 : (w0 + STG) * 128, :]
                            .rearrange("(s p) d -> p s d", p=128),
                            in_=stage[:])

            # ============ replicated-p table all-gather ============
            nc.gpsimd.collective_compute(
                "AllGather",
                mybir.AluOpType.bypass,
                replica_groups=[list(range(N_CORES))],
                ins=[pb2_mine.opt()],
                outs=[pb2_full.opt()],
            )

            # ============ pass 2: conv2 agg + output, per window ============
            with (
                tc.tile_pool(name="g2p0", bufs=9) as g2p0,
                tc.tile_pool(name="g2p1", bufs=9) as g2p1,
                tc.tile_pool(name="opool2", bufs=64) as opool2,
                tc.tile_pool(name="pagg2", bufs=2, space="PSUM") as pagg2,
                tc.tile_pool(name="fsmall", bufs=4) as fsmall,
            ):
                lookup2 = gather_batches(
                    (g2p0, g2p1),
                    (pb2_full[0:half, :], pb2_full[half : 2 * half, :]))
                ohots2 = {g: onehot(opool2, g) for (g, s) in wchunks[0]}
                for w in range(nw):
                    rows = wchunks[w]
                    cur2 = ohots2
                    if w + 1 < nw:
                        ohots2 = {g: onehot(opool2, g)
                                  for (g, s) in wchunks[w + 1]}
                    agg2 = pagg2.tile([128, OUT], F32, tag="agg2")
                    for k, (g, s) in enumerate(rows):
                        gb, j = lookup2[g]
                        o = cur2[g]
                        nc.tensor.matmul(
                            agg2[:], o[:], gb[:, j, 0:OUT],
                            start=(k == 0), stop=(k == len(rows) - 1),
                        )
                    t = fsmall.tile([128, OUT], F32, tag="fo")
                    nc.vector.tensor_tensor(
                        out=t[:], in0=agg2[:], in1=pr_sb[:, w, OUT : 2 * OUT],
                        op=mybir.AluOpType.add)
                    nc.vector.tensor_tensor(
                        out=out_sb[:, w, :], in0=t[:], in1=b2_sb[:],
                        op=mybir.AluOpType.add)
                    if w % 7 == 6:
                        w0 = w - 6
                        nc.sync.dma_start(
                            out=out_d.rearrange(
                                "(w p) c -> p w c", p=128)[:, w0 : w0 + 7, :],
                            in_=out_sb[:, w0 : w0 + 7, :])

    nc.compile()
    return nc


def make_inputs(plan, x, W1l, W1r, b1, Wskip, bskip, gamma, beta, W2l, W2r,
                b2, n_nodes):
    cp, half, npad = plan["cp"], plan["half"], plan["npad"]
    xp = np.zeros((npad, D), np.float32)
    xp[:n_nodes] = np.asarray(x, np.float32)
    wc = np.asarray(W1r, np.float32) + np.asarray(Wskip, np.float32)
    bc = np.asarray(b1, np.float32) + np.asarray(bskip, np.float32)
    wcb = np.concatenate([wc, bc[None, :]], axis=0)
    w2lr_full = np.concatenate(
        [np.asarray(W2l, np.float32), np.asarray(W2r, np.float32)], axis=1)
    w2lr = (w2lr_full.reshape(2, 128, 2 * OUT).transpose(1, 0, 2)
            .reshape(128, 2 * 2 * OUT).copy())
    iota = np.tile(np.arange(128, dtype=np.float32)[None, :], (128, 1))
    ident = np.eye(128, dtype=np.float32)
    i2 = np.zeros((2, D), np.float32)
    i2[0, 0::2] = 1.0
    i2[1, 1::2] = 1.0
    gamma_bc = np.tile(np.asarray(gamma, np.float32)[None, :], (128, 1))
    beta_bc = np.tile(np.asarray(beta, np.float32)[None, :], (128, 1))
    b2_bc = np.tile(np.asarray(b2, np.float32)[None, :], (128, 1))

    common = dict(
        iota=iota, ident=ident, wcb=wcb,
        w1l=np.asarray(W1l, np.float32), w2lr=w2lr,
        gamma_bc=gamma_bc, beta_bc=beta_bc, b2_bc=b2_bc, i2=i2,
    )
    in_maps = []
    for c in range(N_CORES):
        m = dict(common)
        xt = np.ones((D + 1, cp), np.float32)
        xt[:D] = xp[cp * c : cp * (c + 1)].T
        m["xt"] = xt
        m["msg1"] = xp[plan["srcslot"][c]]
        m["gidx"] = plan["gidx_tile"][c]
        m["dstf"] = plan["dstf_tile"][c]
        m["rcf"] = plan["rcf_tile"][c]
        in_maps.append(m)
    return in_maps


_CACHE = {}


def _get_compiled(edge_index, n_nodes):
    key = (edge_index.tobytes()[:512], edge_index.shape, n_nodes)
    if key not in _CACHE:
        plan = make_plan(edge_index, n_nodes)
        nc = build_program(plan)
        _CACHE[key] = (plan, nc)
    return _CACHE[key]


def run(inputs, trace=False):
    x = np.asarray(inputs["x"], np.float32)
    edge_index = np.asarray(inputs["edge_index"], np.int32)
    n_nodes = x.shape[0]
    plan, nc = _get_compiled(edge_index, n_nodes)
    in_maps = make_inputs(
        plan, x, inputs["W1l"], inputs["W1r"], inputs["b1"], inputs["Wskip"],
        inputs["bskip"], inputs["gamma"], inputs["beta"], inputs["W2l"],
        inputs["W2r"], inputs["b2"], n_nodes)
    res = run_bass_kernel_spmd(
        nc, in_maps, list(range(N_CORES)), trace=trace)
    cp = plan["cp"]
    out = np.empty((n_nodes, OUT), np.float32)
    for c in range(N_CORES):
        lo = cp * c
        hi = min(cp * (c + 1), n_nodes)
        out[lo:hi] = res.results[c]["out"][0 : hi - lo]
    return out, res


def kernel(**inputs) -> np.ndarray:
    out, _ = run(inputs)
    return out
